# revision 8
# baseline (speedup 1.0000x reference)
"""Trainium2 Bass kernel for the BVPModel Helmholtz-residual PINN problem.

Computes, for N=131072 collocation points, the residual of a Helmholtz-type
PDE through a 4-256-256-256-2 tanh MLP, including the beta-weighted sum of
diagonal second derivatives w.r.t. the three spatial inputs.

Math: forward-mode second-order jets. Per point, propagate
  - primal       h_l
  - tangents     u_l^d = d h_l / d alpha_d   (scaled by sqrt(beta_d), d=x,y,z)
  - second-order v_l  = sum_d beta_d * (d^2 h_l / d alpha_d^2)  (up to sign)
through the layers:
  affine:  hat = W h,   uhat_d = W u_d,   vhat = W v
  tanh:    z = tanh(hat+b), s = 1 - z^2
           u_d = s * uhat_d
           v   = s * (z * sum_d uhat_d^2 + vhat)        [the -2 is folded into W4v]
Head:
  (pr,pi) = W4 z3 + b4 ;  (qr,qi) = (-2 W4) v3
  kf = ((2 pi (FC f + F0)/C0) * XC*YC*ZC)^2
  res_re = AC*qr + kf*(AC*pr + A0) ;  res_im = BC*qi + kf*(BC*pi + B0)

Sharding: pure data parallel over the 8 NeuronCores (16384 points each),
weights replicated, no collectives. Host pre-transposes weights and
pre-folds constants.
"""

import math
from contextlib import ExitStack

import numpy as np

import concourse.bass as bass
import concourse.bacc as bacc
import concourse.mybir as mybir
import concourse.tile as tile
from concourse.bass_utils import run_bass_kernel_spmd

# ---- problem constants (from reference.py) ----
_C0 = 343.0
FC, F0 = 500.0, 100.0
XC, YC, ZC = 0.7, 0.5, 0.6
AC, A0 = 2.0, 0.1
BC, B0 = 1.5, -0.05
BETA = (
    (YC * ZC) ** 2,  # multiplies p_xx
    (XC * ZC) ** 2,  # multiplies p_yy
    (XC * YC) ** 2,  # multiplies p_zz
)

_GS = np.float32(2.0 * math.pi * FC * (XC * YC * ZC) / _C0)
_GB = np.float32(2.0 * math.pi * F0 * (XC * YC * ZC) / _C0)

N_TOTAL = 131072
N_CORES = 8
NPC = N_TOTAL // N_CORES  # 16384 points per core
H = 256
B = 512                   # points per tile (= max fp32 PSUM bank free dim)
NT = NPC // B             # 32 tiles per core

F32 = mybir.dt.float32
F32R = mybir.dt.float32r
MM_DT = F32R              # matmul operand view dtype

Alu = mybir.AluOpType
Act = mybir.ActivationFunctionType


def _r(ap):
    """View an fp32 AP as float32r for PE (FP22 truncated read, full speed)."""
    return ap.bitcast(MM_DT)


def _build_program():
    """Build the per-core Bass program (identical on all 8 cores)."""
    nc = bacc.Bacc("TRN2", target_bir_lowering=False, debug=False)

    # ---- DRAM I/O ----
    d_xyzf = nc.dram_tensor("xyzf", [4, NPC], F32R, kind="ExternalInput").ap()
    d_fdup = nc.dram_tensor("fdup", [2, NPC], F32, kind="ExternalInput").ap()
    d_w1t = nc.dram_tensor("w1t", [4, H], F32R, kind="ExternalInput").ap()
    d_w2t = nc.dram_tensor("w2t", [H, H], F32R, kind="ExternalInput").ap()
    d_w3t = nc.dram_tensor("w3t", [H, H], F32R, kind="ExternalInput").ap()
    d_w4t = nc.dram_tensor("w4t", [H, 2], F32R, kind="ExternalInput").ap()
    d_w4vt = nc.dram_tensor("w4vt", [H, 2], F32R, kind="ExternalInput").ap()
    d_bias = nc.dram_tensor("bias", [H, 3], F32, kind="ExternalInput").ap()
    d_consts = nc.dram_tensor("consts", [H, 4], F32, kind="ExternalInput").ap()
    d_hconst = nc.dram_tensor("hconst", [2, 2], F32, kind="ExternalInput").ap()
    d_out = nc.dram_tensor("out", [2, NPC], F32, kind="ExternalOutput").ap()

    gs = 2.0 * math.pi * FC * (XC * YC * ZC) / _C0
    gb = 2.0 * math.pi * F0 * (XC * YC * ZC) / _C0

    with tile.TileContext(nc) as tc, ExitStack() as ctx:
        singles = ctx.enter_context(tc.tile_pool(name="singles", bufs=1))
        work = ctx.enter_context(tc.tile_pool(name="work", bufs=2))
        psum = ctx.enter_context(tc.tile_pool(name="psum", bufs=8, space="PSUM"))

        # ---- load weights / constants once ----
        w1t = singles.tile([4, H], F32R)
        nc.sync.dma_start(out=w1t, in_=d_w1t)
        w2t = [singles.tile([128, H], F32R, name=f"w2t{k}") for k in range(2)]
        w3t = [singles.tile([128, H], F32R, name=f"w3t{k}") for k in range(2)]
        w4t = [singles.tile([128, 2], F32R, name=f"w4t{k}") for k in range(2)]
        w4vt = [singles.tile([128, 2], F32R, name=f"w4vt{k}") for k in range(2)]
        bias = [singles.tile([128, 3], F32, name=f"bias{k}") for k in range(2)]
        cst = [singles.tile([128, 4], F32, name=f"cst{k}") for k in range(2)]
        for k in range(2):
            sl = slice(k * 128, (k + 1) * 128)
            nc.sync.dma_start(out=w2t[k], in_=d_w2t[sl, :])
            nc.sync.dma_start(out=w3t[k], in_=d_w3t[sl, :])
            nc.sync.dma_start(out=w4t[k], in_=d_w4t[sl, :])
            nc.sync.dma_start(out=w4vt[k], in_=d_w4vt[sl, :])
            nc.sync.dma_start(out=bias[k], in_=d_bias[sl, :])
            nc.sync.dma_start(out=cst[k], in_=d_consts[sl, :])
        hconst = singles.tile([2, 2], F32)
        nc.sync.dma_start(out=hconst, in_=d_hconst)
        acbc = hconst[:, 0:1]
        hb4 = hconst[:, 1:2]

        for j in range(NT):
            js = slice(j * B, (j + 1) * B)

            xyzf = work.tile([4, B], F32R, name="xyzf")
            nc.sync.dma_start(out=xyzf, in_=d_xyzf[:, js])
            fd = work.tile([2, B], F32, name="fd")
            nc.sync.dma_start(out=fd, in_=d_fdup[:, js])

            # ---------- layer 1 ----------
            z1, s1, v1 = [], [], []
            u1 = [[None, None] for _ in range(3)]
            for c in range(2):
                pa = psum.tile([128, B], F32, tag="ps", name=f"pa1_{c}")
                nc.tensor.matmul(
                    pa, w1t[:, c * 128:(c + 1) * 128], xyzf,
                    start=True, stop=True,
                )
                z = work.tile([128, B], F32R, name=f"z1_{c}")
                nc.scalar.activation(z, pa, Act.Tanh, bias=bias[c][:, 0:1])
                zq = work.tile([128, B], F32, name=f"zq1_{c}", bufs=1)
                nc.scalar.activation(zq, z.bitcast(F32), Act.Square)
                s = work.tile([128, B], F32, name=f"s1_{c}")
                nc.vector.tensor_scalar(s, zq, -1.0, 1.0, Alu.mult, Alu.add)
                for d in range(3):
                    u = work.tile([128, B], F32R, name=f"u1_{d}_{c}")
                    nc.vector.tensor_scalar(u, s, cst[c][:, d:d + 1], None, Alu.mult)
                    u1[d][c] = u
                v = work.tile([128, B], F32R, name=f"v1_{c}")
                nc.vector.scalar_tensor_tensor(
                    v, z.bitcast(F32), cst[c][:, 3:4], s, op0=Alu.mult, op1=Alu.mult
                )
                z1.append(z)
                s1.append(s)
                v1.append(v)

            # ---------- layers 2 and 3 ----------
            def tanh_layer(wt, bcol, hin, uin, vin, lname, need_u):
                """One hidden tanh layer of the jet propagation.

                hin/uin[d]/vin: per-chunk SBUF tiles of the incoming primal,
                tangent, and second-order channels. Returns (z, u_out, v_out).
                """
                zs, ss, vs = [], [], []
                us = [[None, None] for _ in range(3)] if need_u else None
                for m in range(2):
                    msl = slice(m * 128, (m + 1) * 128)
                    ph = psum.tile([128, B], F32, tag="ps", name=f"{lname}_ph{m}")
                    for k in range(2):
                        nc.tensor.matmul(ph, wt[k][:, msl], hin[k],
                                         start=(k == 0), stop=(k == 1))
                    pu = []
                    for d in range(3):
                        p = psum.tile([128, B], F32, tag="ps", name=f"{lname}_pu{d}{m}")
                        for k in range(2):
                            nc.tensor.matmul(p, wt[k][:, msl], uin[d][k],
                                             start=(k == 0), stop=(k == 1))
                        pu.append(p)
                    pv = psum.tile([128, B], F32, tag="ps", name=f"{lname}_pv{m}")
                    for k in range(2):
                        nc.tensor.matmul(pv, wt[k][:, msl], vin[k],
                                         start=(k == 0), stop=(k == 1))

                    z = work.tile([128, B], F32R, name=f"{lname}_z{m}")
                    nc.scalar.activation(z, ph, Act.Tanh, bias=bcol[m])
                    zq = work.tile([128, B], F32, name=f"{lname}_zq{m}", bufs=1)
                    nc.scalar.activation(zq, z.bitcast(F32), Act.Square)
                    s = work.tile([128, B], F32, name=f"{lname}_s{m}")
                    nc.vector.tensor_scalar(s, zq, -1.0, 1.0, Alu.mult, Alu.add)

                    y = []
                    for d in range(3):
                        t = work.tile([128, B], F32, name=f"{lname}_y{d}{m}", bufs=1)
                        nc.scalar.activation(t, pu[d], Act.Square)
                        y.append(t)
                    if need_u:
                        for d in range(3):
                            u = work.tile([128, B], F32R, name=f"{lname}_u{d}{m}")
                            nc.vector.tensor_mul(u, pu[d], s)
                            us[d][m] = u
                    # q-chain in one temp, in-place:
                    #   t = (y0+y1+y2) * z + vhat ; v = t * s
                    tq = work.tile([128, B], F32, name=f"{lname}_t{m}", bufs=1)
                    nc.vector.tensor_add(tq, y[0], y[1])
                    nc.vector.tensor_add(tq, tq, y[2])
                    nc.vector.tensor_mul(tq, tq, z.bitcast(F32))
                    nc.vector.scalar_tensor_tensor(
                        tq, tq, 1.0, pv, op0=Alu.mult, op1=Alu.add
                    )
                    v = work.tile([128, B], F32R, name=f"{lname}_v{m}")
                    nc.vector.tensor_mul(v, tq, s)
                    zs.append(z)
                    ss.append(s)
                    vs.append(v)
                return zs, us, vs

            z2, u2, v2 = tanh_layer(
                w2t, [bias[m][:, 1:2] for m in range(2)],
                z1, u1, v1, "l2", need_u=True,
            )
            z3, _, v3 = tanh_layer(
                w3t, [bias[m][:, 2:3] for m in range(2)],
                z2, u2, v2, "l3", need_u=False,
            )

            # ---------- head ----------
            p1 = psum.tile([2, B], F32, tag="ps", name="p1")
            p2 = psum.tile([2, B], F32, tag="ps", name="p2")
            for k in range(2):
                nc.tensor.matmul(p1, w4t[k], z3[k],
                                 start=(k == 0), stop=(k == 1))
            for k in range(2):
                nc.tensor.matmul(p2, w4vt[k], v3[k],
                                 start=(k == 0), stop=(k == 1))

            # fdup is host-prepared as gs*f + gb, so kf = Square(fdup)
            kf = work.tile([2, B], F32, name="kf", bufs=1)
            nc.scalar.activation(kf, fd, Act.Square)
            m1 = work.tile([2, B], F32, name="m1", bufs=1)
            nc.vector.tensor_scalar(m1, p1, acbc, hb4, Alu.mult, Alu.add)
            m2 = work.tile([2, B], F32, name="m2", bufs=1)
            nc.vector.tensor_mul(m2, m1, kf)
            res = work.tile([2, B], F32, name="res")
            nc.vector.scalar_tensor_tensor(
                res, p2, acbc, m2, op0=Alu.mult, op1=Alu.add
            )
            nc.sync.dma_start(out=d_out[:, js], in_=res)

    nc.compile()
    return nc


def _host_prep(inputs):
    """Host-side preprocessing: shard points, transpose weights, fold consts."""
    x = np.ascontiguousarray(np.asarray(inputs["x"], np.float32))
    y = np.ascontiguousarray(np.asarray(inputs["y"], np.float32))
    z = np.ascontiguousarray(np.asarray(inputs["z"], np.float32))
    f = np.ascontiguousarray(np.asarray(inputs["f"], np.float32))
    W1 = np.asarray(inputs["W1"], np.float32)
    b1 = np.asarray(inputs["b1"], np.float32)
    W2 = np.asarray(inputs["W2"], np.float32)
    b2 = np.asarray(inputs["b2"], np.float32)
    W3 = np.asarray(inputs["W3"], np.float32)
    b3 = np.asarray(inputs["b3"], np.float32)
    W4 = np.asarray(inputs["W4"], np.float32)
    b4 = np.asarray(inputs["b4"], np.float32)

    sb = np.sqrt(np.asarray(BETA, np.float64))
    chat = (sb[None, :] * W1[:, :3].astype(np.float64)).astype(np.float32)
    cc = (np.asarray(BETA)[None, :] * W1[:, :3].astype(np.float64) ** 2).sum(1)
    consts = np.concatenate([chat, cc.astype(np.float32)[:, None]], axis=1)
    consts = np.ascontiguousarray(consts)                       # [256, 4]

    biasm = np.ascontiguousarray(np.stack([b1, b2, b3], axis=1))  # [256, 3]
    w1t = np.ascontiguousarray(W1.T)                            # [4, 256]
    w2t = np.ascontiguousarray(W2.T)                            # [256, 256]
    w3t = np.ascontiguousarray(W3.T)
    w4t = np.ascontiguousarray(W4.T)                            # [256, 2]
    w4vt = np.ascontiguousarray((-2.0 * W4).T)
    hconst = np.array(
        [[AC, AC * b4[0] + A0], [BC, BC * b4[1] + B0]], np.float32
    )                                                           # [2, 2]

    xyzf = np.stack([x, y, z, f])                               # [4, N]
    in_maps = []
    for c in range(N_CORES):
        cs = slice(c * NPC, (c + 1) * NPC)
        in_maps.append({
            "xyzf": np.ascontiguousarray(xyzf[:, cs]),
            "fdup": np.ascontiguousarray(
                np.broadcast_to(_GS * f[cs] + _GB, (2, NPC))
            ),
            "w1t": w1t, "w2t": w2t, "w3t": w3t,
            "w4t": w4t, "w4vt": w4vt,
            "bias": biasm, "consts": consts, "hconst": hconst,
        })
    return in_maps


_NC_CACHE = None


def get_program():
    global _NC_CACHE
    if _NC_CACHE is None:
        _NC_CACHE = _build_program()
    return _NC_CACHE


def kernel(**inputs) -> np.ndarray:
    nc = get_program()
    in_maps = _host_prep(inputs)
    r = run_bass_kernel_spmd(nc, in_maps, core_ids=list(range(N_CORES)))
    return np.concatenate([r.results[c]["out"] for c in range(N_CORES)], axis=1)


# revision 9
# speedup vs baseline: 1.2927x; 1.2927x over previous
"""Trainium2 Bass kernel for the BVPModel Helmholtz-residual PINN problem.

Computes, for N=131072 collocation points, the residual of a Helmholtz-type
PDE through a 4-256-256-256-2 tanh MLP, including the beta-weighted sum of
diagonal second derivatives w.r.t. the three spatial inputs.

Math: forward-mode second-order jets (forward Laplacian). Per point:
  - primal       h_l
  - tangents     u_l^d (scaled by sqrt(beta_d), d=x,y,z)
  - second-order v_l = -1/2 sum_d beta_d d^2h/dalpha_d^2
through the layers:
  affine:  hat = W h,   uhat_d = W u_d,   vhat = W v
  tanh:    z = tanh(hat+b), s = 1 - z^2
           u_d = s * uhat_d
           v   = z * sum_d uhat_d^2 + vhat, times s    [-2 folded into W4v]
Layer-1 tangent specialization: u1_d = s1 * chat_d with chat_d a constant
vector, so uhat2_d = (W2 diag(chat_d)) @ s1 — the diag is folded into three
host-precomputed weight matrices Wc_d and the layer-1 tangent elementwise
ops vanish.

Precision: fp16 matmul operands / elementwise (error ~2e-3 of scale);
L1 matmul in fp32r to avoid rounding the input coordinates; PSUM fp32.

Sharding: pure data parallel over 8 NeuronCores (16384 points each),
weights replicated, no collectives.
"""

import math
from contextlib import ExitStack

import numpy as np

import concourse.bass as bass
import concourse.bacc as bacc
import concourse.mybir as mybir
import concourse.tile as tile
from concourse.bass_utils import run_bass_kernel_spmd

# ---- problem constants (from the BVPModel definition) ----
_C0 = 343.0
FC, F0 = 500.0, 100.0
XC, YC, ZC = 0.7, 0.5, 0.6
AC, A0 = 2.0, 0.1
BC, B0 = 1.5, -0.05
BETA = ((YC * ZC) ** 2, (XC * ZC) ** 2, (XC * YC) ** 2)

_GS = np.float32(2.0 * math.pi * FC * (XC * YC * ZC) / _C0)
_GB = np.float32(2.0 * math.pi * F0 * (XC * YC * ZC) / _C0)

N_TOTAL = 131072
N_CORES = 8
NPC = N_TOTAL // N_CORES  # 16384 points per core
H = 256
B = 512                   # points per tile
NT = NPC // B             # tiles per core

F32 = mybir.dt.float32
F32R = mybir.dt.float32r
F16 = mybir.dt.float16

Alu = mybir.AluOpType
Act = mybir.ActivationFunctionType


def _build_program():
    """Build the per-core Bass program (identical on all 8 cores)."""
    nc = bacc.Bacc("TRN2", target_bir_lowering=False, debug=False)

    # ---- DRAM I/O ----
    d_xyzf = nc.dram_tensor("xyzf", [4, NPC], F32R, kind="ExternalInput").ap()
    d_fdup = nc.dram_tensor("fdup", [2, NPC], F16, kind="ExternalInput").ap()
    d_w1t = nc.dram_tensor("w1t", [4, H], F32R, kind="ExternalInput").ap()
    d_w2t = nc.dram_tensor("w2t", [H, H], F16, kind="ExternalInput").ap()
    d_w3t = nc.dram_tensor("w3t", [H, H], F16, kind="ExternalInput").ap()
    d_wct = [
        nc.dram_tensor(f"wct{d}", [H, H], F16, kind="ExternalInput").ap()
        for d in range(3)
    ]
    d_w4t = nc.dram_tensor("w4t", [H, 2], F16, kind="ExternalInput").ap()
    d_w4vt = nc.dram_tensor("w4vt", [H, 2], F16, kind="ExternalInput").ap()
    d_bias = nc.dram_tensor("bias", [H, 3], F32, kind="ExternalInput").ap()
    d_cc = nc.dram_tensor("cc", [H, 1], F32, kind="ExternalInput").ap()
    d_hconst = nc.dram_tensor("hconst", [2, 2], F32, kind="ExternalInput").ap()
    d_out = nc.dram_tensor("out", [2, NPC], F32, kind="ExternalOutput").ap()

    with tile.TileContext(nc) as tc, ExitStack() as ctx:
        singles = ctx.enter_context(tc.tile_pool(name="singles", bufs=1))
        work = ctx.enter_context(tc.tile_pool(name="work", bufs=2))
        psum = ctx.enter_context(tc.tile_pool(name="psum", bufs=8, space="PSUM"))

        # ---- load weights / constants once ----
        w1t = singles.tile([4, H], F32R)
        nc.sync.dma_start(out=w1t, in_=d_w1t)
        w2t = [singles.tile([128, H], F16, name=f"w2t{k}") for k in range(2)]
        w3t = [singles.tile([128, H], F16, name=f"w3t{k}") for k in range(2)]
        wct = [[singles.tile([128, H], F16, name=f"wct{d}_{k}") for k in range(2)]
               for d in range(3)]
        w4t = [singles.tile([128, 2], F16, name=f"w4t{k}") for k in range(2)]
        w4vt = [singles.tile([128, 2], F16, name=f"w4vt{k}") for k in range(2)]
        bias = [singles.tile([128, 3], F32, name=f"bias{k}") for k in range(2)]
        cc = [singles.tile([128, 1], F32, name=f"cc{k}") for k in range(2)]
        for k in range(2):
            sl = slice(k * 128, (k + 1) * 128)
            nc.sync.dma_start(out=w2t[k], in_=d_w2t[sl, :])
            nc.sync.dma_start(out=w3t[k], in_=d_w3t[sl, :])
            for d in range(3):
                nc.sync.dma_start(out=wct[d][k], in_=d_wct[d][sl, :])
            nc.sync.dma_start(out=w4t[k], in_=d_w4t[sl, :])
            nc.sync.dma_start(out=w4vt[k], in_=d_w4vt[sl, :])
            nc.sync.dma_start(out=bias[k], in_=d_bias[sl, :])
            nc.sync.dma_start(out=cc[k], in_=d_cc[sl, :])
        hconst = singles.tile([2, 2], F32)
        nc.sync.dma_start(out=hconst, in_=d_hconst)
        acbc = hconst[:, 0:1]
        hb4 = hconst[:, 1:2]

        for j in range(NT):
            js = slice(j * B, (j + 1) * B)

            xyzf = work.tile([4, B], F32R, name="xyzf")
            nc.sync.dma_start(out=xyzf, in_=d_xyzf[:, js])
            fd = work.tile([2, B], F16, name="fd")
            nc.sync.dma_start(out=fd, in_=d_fdup[:, js])

            # ---------- layer 1 (fp32r matmul; elementwise fp16 out) ----------
            z1, s1, v1 = [], [], []
            for c in range(2):
                pa = psum.tile([128, B], F32, tag="ps", name=f"pa1_{c}")
                nc.tensor.matmul(
                    pa, w1t[:, c * 128:(c + 1) * 128], xyzf,
                    start=True, stop=True,
                )
                z = work.tile([128, B], F16, name=f"z1_{c}")
                nc.scalar.activation(z, pa, Act.Tanh, bias=bias[c][:, 0:1])
                zq = work.tile([128, B], F16, name=f"zq1_{c}", bufs=1)
                nc.vector.tensor_mul(zq, z, z)
                s = work.tile([128, B], F16, name=f"s1_{c}")
                nc.vector.tensor_scalar(s, zq, -1.0, 1.0, Alu.mult, Alu.add)
                v = work.tile([128, B], F16, name=f"v1_{c}")
                nc.vector.scalar_tensor_tensor(
                    v, z, cc[c], s, op0=Alu.mult, op1=Alu.mult
                )
                z1.append(z)
                s1.append(s)
                v1.append(v)

            # ---------- layers 2 and 3 ----------
            def tanh_layer(wt, wut, bcol, hin, uin, vin, lname, need_u):
                """One tanh layer of the jet propagation.

                wt: weight chunks for the h/v channels; wut[d]: weight chunks
                for tangent direction d (Wc_d at layer 2, W3 at layer 3).
                uin[d]: per-chunk rhs tiles of tangent direction d.
                """
                zs, ss, vs = [], [], []
                us = [[None, None] for _ in range(3)] if need_u else None
                for m in range(2):
                    msl = slice(m * 128, (m + 1) * 128)
                    ph = psum.tile([128, B], F32, tag="ps", name=f"{lname}_ph{m}")
                    for k in range(2):
                        nc.tensor.matmul(ph, wt[k][:, msl], hin[k],
                                         start=(k == 0), stop=(k == 1))
                    pu = []
                    for d in range(3):
                        p = psum.tile([128, B], F32, tag="ps",
                                      name=f"{lname}_pu{d}{m}")
                        for k in range(2):
                            nc.tensor.matmul(p, wut[d][k][:, msl], uin[d][k],
                                             start=(k == 0), stop=(k == 1))
                        pu.append(p)
                    pv = psum.tile([128, B], F32, tag="ps", name=f"{lname}_pv{m}")
                    for k in range(2):
                        nc.tensor.matmul(pv, wt[k][:, msl], vin[k],
                                         start=(k == 0), stop=(k == 1))

                    z = work.tile([128, B], F16, name=f"{lname}_z{m}")
                    nc.scalar.activation(z, ph, Act.Tanh, bias=bcol[m])
                    zq = work.tile([128, B], F16, name=f"{lname}_zq{m}", bufs=1)
                    nc.vector.tensor_mul(zq, z, z)
                    s = work.tile([128, B], F16, name=f"{lname}_s{m}")
                    nc.vector.tensor_scalar(s, zq, -1.0, 1.0, Alu.mult, Alu.add)

                    y = []
                    for d in range(3):
                        t = work.tile([128, B], F16, name=f"{lname}_y{d}{m}",
                                      bufs=1)
                        nc.scalar.activation(t, pu[d], Act.Square)
                        y.append(t)
                    if need_u:
                        for d in range(3):
                            u = work.tile([128, B], F16, name=f"{lname}_u{d}{m}")
                            nc.vector.tensor_mul(u, pu[d], s)
                            us[d][m] = u
                    # q-chain: tq = (y0+y1+y2)*z + vhat ; v = tq*s
                    tq = work.tile([128, B], F16, name=f"{lname}_t{m}", bufs=1)
                    nc.vector.tensor_add(tq, y[0], y[1])
                    nc.vector.tensor_add(tq, tq, y[2])
                    nc.vector.tensor_mul(tq, tq, z)
                    tq2 = work.tile([128, B], F16, name=f"{lname}_t2{m}", bufs=1)
                    nc.vector.scalar_tensor_tensor(
                        tq2, tq, 1.0, pv, op0=Alu.mult, op1=Alu.add
                    )
                    v = work.tile([128, B], F16, name=f"{lname}_v{m}")
                    nc.vector.tensor_mul(v, tq2, s)
                    zs.append(z)
                    ss.append(s)
                    vs.append(v)
                return zs, us, vs

            # layer 2: tangent rhs is s1 (chat folded into Wc weights)
            z2, u2, v2 = tanh_layer(
                w2t, wct, [bias[m][:, 1:2] for m in range(2)],
                z1, [[s1, s1, s1][d] for d in range(3)], v1, "l2", need_u=True,
            )
            z3, _, v3 = tanh_layer(
                w3t, [w3t, w3t, w3t], [bias[m][:, 2:3] for m in range(2)],
                z2, u2, v2, "l3", need_u=False,
            )

            # ---------- head ----------
            p1 = psum.tile([2, B], F32, tag="ps", name="p1")
            p2 = psum.tile([2, B], F32, tag="ps", name="p2")
            for k in range(2):
                nc.tensor.matmul(p1, w4t[k], z3[k],
                                 start=(k == 0), stop=(k == 1))
            for k in range(2):
                nc.tensor.matmul(p2, w4vt[k], v3[k],
                                 start=(k == 0), stop=(k == 1))

            # res = acbc*q + fd^2 * (acbc*p + hb4), fd = gs*f+gb (host)
            m1 = work.tile([2, B], F16, name="m1", bufs=1)
            nc.vector.tensor_scalar(m1, p1, acbc, hb4, Alu.mult, Alu.add)
            t1 = work.tile([2, B], F16, name="t1", bufs=1)
            nc.vector.tensor_mul(t1, m1, fd)
            m2 = work.tile([2, B], F16, name="m2", bufs=1)
            nc.vector.tensor_mul(m2, t1, fd)
            res = work.tile([2, B], F32, name="res")
            nc.vector.scalar_tensor_tensor(
                res, p2, acbc, m2, op0=Alu.mult, op1=Alu.add
            )
            nc.sync.dma_start(out=d_out[:, js], in_=res)

    nc.compile()
    return nc


def _host_prep(inputs):
    """Host-side preprocessing: shard points, transpose weights, fold consts."""
    x = np.asarray(inputs["x"], np.float32)
    y = np.asarray(inputs["y"], np.float32)
    z = np.asarray(inputs["z"], np.float32)
    f = np.asarray(inputs["f"], np.float32)
    W1 = np.asarray(inputs["W1"], np.float32)
    b1 = np.asarray(inputs["b1"], np.float32)
    W2 = np.asarray(inputs["W2"], np.float32)
    b2 = np.asarray(inputs["b2"], np.float32)
    W3 = np.asarray(inputs["W3"], np.float32)
    b3 = np.asarray(inputs["b3"], np.float32)
    W4 = np.asarray(inputs["W4"], np.float32)
    b4 = np.asarray(inputs["b4"], np.float32)

    sb = np.sqrt(np.asarray(BETA, np.float64))
    chat = (sb[None, :] * W1[:, :3].astype(np.float64)).astype(np.float32)
    cc = (np.asarray(BETA)[None, :] * W1[:, :3].astype(np.float64) ** 2) \
        .sum(1).astype(np.float32)[:, None]                     # [256, 1]

    biasm = np.ascontiguousarray(np.stack([b1, b2, b3], axis=1))  # [256, 3]
    w1t = np.ascontiguousarray(W1.T)                            # [4, 256] f32
    w2t = np.ascontiguousarray(W2.T.astype(np.float16))
    w3t = np.ascontiguousarray(W3.T.astype(np.float16))
    wct = [
        np.ascontiguousarray((W2 * chat[None, :, d]).T.astype(np.float16))
        for d in range(3)
    ]
    w4t = np.ascontiguousarray(W4.T.astype(np.float16))         # [256, 2]
    w4vt = np.ascontiguousarray((-2.0 * W4).T.astype(np.float16))
    hconst = np.array(
        [[AC, AC * b4[0] + A0], [BC, BC * b4[1] + B0]], np.float32
    )

    xyzf = np.stack([x, y, z, f])                               # [4, N]
    fd_full = (_GS * f + _GB).astype(np.float16)
    in_maps = []
    for c in range(N_CORES):
        cs = slice(c * NPC, (c + 1) * NPC)
        im = {
            "xyzf": np.ascontiguousarray(xyzf[:, cs]),
            "fdup": np.ascontiguousarray(
                np.broadcast_to(fd_full[cs], (2, NPC))
            ),
            "w1t": w1t, "w2t": w2t, "w3t": w3t,
            "w4t": w4t, "w4vt": w4vt,
            "bias": biasm, "cc": cc, "hconst": hconst,
        }
        for d in range(3):
            im[f"wct{d}"] = wct[d]
        in_maps.append(im)
    return in_maps


_NC_CACHE = None


def get_program():
    global _NC_CACHE
    if _NC_CACHE is None:
        _NC_CACHE = _build_program()
    return _NC_CACHE


def kernel(**inputs) -> np.ndarray:
    nc = get_program()
    in_maps = _host_prep(inputs)
    r = run_bass_kernel_spmd(nc, in_maps, core_ids=list(range(N_CORES)))
    return np.concatenate([r.results[c]["out"] for c in range(N_CORES)], axis=1)
